# revision 3
# baseline (speedup 1.0000x reference)
"""Trainium2 Bass kernel for the DagnabbitAutoEncoder DAG scan.

Strategy: the scan over 65536 trunk nodes is level-scheduled. Node depth =
1 + max(parent depths); the random DAG has only ~28 levels, and all nodes of
one level are independent.  Work is sharded across the 8 NeuronCores by node
TYPE (type t -> cores 2t, 2t+1), so each core's per-level shard uses a single
MLP weight set and its matmuls get a long free dimension.

Per level, per core:
  indirect-DMA gather of the 2 parent embeddings for each shard node (from a
  DRAM embeddings buffer held in level-permuted order), PE-transpose to
  feature-major, 2-layer MLP (float32r matmuls = TF32, exact-erf Gelu on the
  ACT engine), PE-transpose back to node-major, then (a) indirect-DMA scatter
  of the shard's embeddings into this core's external output at the original
  row ids and (b) AllGather of the level block into every core's DRAM buffer
  so the next level can gather from it.

The host merges the 8 partial outputs (each core owns its scattered rows).
"""

import math
import os

import numpy as np

R = 256
D = 256
IN_DEG = 2
NCORES = 8
P = 128
PSUM_N = 512


# ---------------------------------------------------------------------------
# host-side preprocessing
# ---------------------------------------------------------------------------

def _compute_levels(idx):
    n = idx.shape[0]
    depth = np.zeros(R + n, np.int32)
    ia = idx[:, 0]
    ib = idx[:, 1]
    d = depth
    for i in range(n):
        da = d[ia[i]]
        db = d[ib[i]]
        d[R + i] = (da if da > db else db) + 1
    return depth[R:]


def _plan(idx, types):
    """Level/shard plan. Returns dict with per-level k, per-core tables."""
    n = idx.shape[0]
    lv = _compute_levels(idx)
    L = int(lv.max()) if n else 0
    order = np.argsort(lv, kind="stable")

    pos = np.zeros(R + n, np.int64)  # permuted buffer row of each orig row
    pos[:R] = np.arange(R)
    slots = R
    level_ks = []
    shards = []  # per level: list of 8 arrays of orig node ids
    lo = 0
    lv_sorted = lv[order]
    for l in range(1, L + 1):
        hi = lo + np.searchsorted(lv_sorted[lo:], l + 1)
        nodes = order[lo:hi]
        lo = hi
        per_core = []
        for t in range(4):
            nt = nodes[types[nodes] == t]
            per_core.append(nt[0::2])
            per_core.append(nt[1::2])
        n_pad = max(len(s) for s in per_core)
        k = max(1, math.ceil(n_pad / P))
        pad = P * k
        for c in range(NCORES):
            s = per_core[c]
            pos[R + s] = slots + c * pad + np.arange(len(s))
        level_ks.append(k)
        shards.append(per_core)
        slots += NCORES * pad

    K = sum(level_ks)
    gidx = np.zeros((NCORES, P, 2 * K), np.int32)
    src_rows = [[] for _ in range(NCORES)]  # rows in shard_out
    dst_rows = [[] for _ in range(NCORES)]  # rows in final output
    goff = 0
    soff = 0
    for l in range(L):
        k = level_ks[l]
        for c in range(NCORES):
            s = shards[l][c]
            m = len(s)
            sl = np.arange(m)
            pp = sl % P
            jj = sl // P
            gidx[c, pp, goff + 2 * jj] = pos[idx[s, 0]]
            gidx[c, pp, goff + 2 * jj + 1] = pos[idx[s, 1]]
            src_rows[c].append(soff * P + sl)
            dst_rows[c].append(R + s)
        goff += 2 * k
        soff += k
    src_rows = [np.concatenate(o) if o else np.zeros(0, np.int64) for o in src_rows]
    dst_rows = [np.concatenate(o) if o else np.zeros(0, np.int64) for o in dst_rows]
    return {
        "level_ks": level_ks,
        "slots": slots,
        "K": K,
        "gidx": gidx,
        "src_rows": src_rows,
        "dst_rows": dst_rows,
    }


# ---------------------------------------------------------------------------
# Bass program
# ---------------------------------------------------------------------------

def _build_program(level_ks, slots, K, out_rows):
    import concourse.bass as bass
    import concourse.tile as tile
    from concourse import bacc, mybir
    from concourse.masks import make_identity

    F32 = mybir.dt.float32
    F32R = mybir.dt.float32r
    I32 = mybir.dt.int32
    AF = mybir.ActivationFunctionType

    nc = bacc.Bacc("TRN2", target_bir_lowering=False, debug=False,
                   num_devices=NCORES)
    t_root = nc.dram_tensor("roots", [R, D], F32, kind="ExternalInput")
    t_w1 = nc.dram_tensor("w1", [P, 4 * 512], F32, kind="ExternalInput")
    t_w2 = nc.dram_tensor("w2", [P, 4 * 256], F32, kind="ExternalInput")
    t_b1 = nc.dram_tensor("b1", [P, 4], F32, kind="ExternalInput")
    t_b2 = nc.dram_tensor("b2", [P, 2], F32, kind="ExternalInput")
    t_gidx = nc.dram_tensor("gidx", [P, 2 * K], I32, kind="ExternalInput")
    t_sout = nc.dram_tensor("shard_out", [P * K, D], F32, kind="ExternalOutput")
    buffer = nc.dram_tensor("buffer", [slots, D], F32, kind="Internal",
                            addr_space="Shared")
    groups = [list(range(NCORES))]
    k_max = max(level_ks)

    with tile.TileContext(nc) as tc:
        with (
            tc.tile_pool(name="const", bufs=1) as constp,
            tc.tile_pool(name="sbuf", bufs=1) as sbufp,
            tc.tile_pool(name="psum", bufs=1, space="PSUM") as psump,
            tc.tile_pool(name="dram", bufs=2, space="DRAM") as dramp,
        ):
            ident = constp.tile([P, P], F32)
            make_identity(nc, ident[:])
            w1_sb = constp.tile([P, 4 * 512], F32R)
            nc.sync.dma_start(w1_sb[:], t_w1[:].bitcast(F32R))
            w2_sb = constp.tile([P, 4 * 256], F32R)
            nc.sync.dma_start(w2_sb[:], t_w2[:].bitcast(F32R))
            b1_sb = constp.tile([P, 4], F32)
            nc.sync.dma_start(b1_sb[:], t_b1[:])
            b2_sb = constp.tile([P, 2], F32)
            nc.sync.dma_start(b2_sb[:], t_b2[:])
            gidx_sb = constp.tile([P, 2 * K], I32)
            nc.sync.dma_start(gidx_sb[:], t_gidx[:])

            # roots -> buffer[0:R]
            stg = sbufp.tile([P, (R // P) * D], F32, tag="stg")
            nc.sync.dma_start(
                stg[:], t_root[:].rearrange("(j p) d -> p j d", p=P))
            nc.sync.dma_start(
                buffer[0:R, :].rearrange("(j p) d -> p j d", p=P), stg[:])

            goff = 0
            soff = 0
            blk = R
            for l, k in enumerate(level_ks):
                npad = P * k
                gx = sbufp.tile([P, 2 * k_max * D], F32, tag="gx",
                                bufs=2, name="gx")[:, : 2 * k * D]
                for col in range(2 * k):
                    nc.gpsimd.indirect_dma_start(
                        out=gx[:, col * D:(col + 1) * D], out_offset=None,
                        in_=buffer[:],
                        in_offset=bass.IndirectOffsetOnAxis(
                            ap=gidx_sb[:, goff + col: goff + col + 1], axis=0))

                xt = [sbufp.tile([P, P * k_max], F32R, tag=f"xt{ic}",
                                 name=f"xt{ic}")[:, : npad] for ic in range(4)]
                for j in range(k):
                    for ic in range(4):
                        tp = psump.tile([P, P], F32, tag="tpose", bufs=2,
                                        name="tp")
                        nc.tensor.transpose(
                            tp[:],
                            gx[:, 2 * j * D + ic * P: 2 * j * D + (ic + 1) * P],
                            ident[:])
                        nc.vector.tensor_copy(xt[ic][:, j * P:(j + 1) * P],
                                              tp[:])

                h_sb = [sbufp.tile([P, P * k_max], F32R, tag=f"h{oc}",
                                   name=f"h{oc}")[:, : npad] for oc in range(4)]
                et_sb = [sbufp.tile([P, P * k_max], F32, tag=f"et{o2}",
                                    name=f"et{o2}")[:, : npad] for o2 in range(2)]
                for g in range(math.ceil(npad / PSUM_N)):
                    c0 = g * PSUM_N
                    ng = min(PSUM_N, npad - c0)
                    cols = slice(c0, c0 + ng)
                    for oc in range(4):
                        hp = psump.tile([P, PSUM_N], F32, tag=f"hp{oc % 2}",
                                        name="hp")[:, :ng]
                        for ic in range(4):
                            nc.tensor.matmul(
                                hp,
                                lhsT=w1_sb[:, ic * 512 + oc * P:
                                           ic * 512 + (oc + 1) * P],
                                rhs=xt[ic][:, cols],
                                start=(ic == 0), stop=(ic == 3))
                        nc.scalar.activation(h_sb[oc][:, cols], hp, AF.Gelu,
                                             bias=b1_sb[:, oc:oc + 1])
                    for o2 in range(2):
                        ep = psump.tile([P, PSUM_N], F32, tag=f"ep{o2}",
                                        name="ep")[:, :ng]
                        for ic in range(4):
                            nc.tensor.matmul(
                                ep,
                                lhsT=w2_sb[:, ic * 256 + o2 * P:
                                           ic * 256 + (o2 + 1) * P],
                                rhs=h_sb[ic][:, cols],
                                start=(ic == 0), stop=(ic == 3))
                        nc.vector.tensor_add(
                            et_sb[o2][:, cols], ep,
                            b2_sb[:, o2:o2 + 1].to_broadcast([P, ng]))

                e_sb = sbufp.tile([P, k_max * D], F32, tag="e",
                                  name="e")[:, : k * D]
                for j in range(k):
                    for o2 in range(2):
                        tp = psump.tile([P, P], F32, tag="tpose", bufs=2,
                                        name="tp")
                        nc.tensor.transpose(
                            tp[:], et_sb[o2][:, j * P:(j + 1) * P], ident[:])
                        nc.vector.tensor_copy(
                            e_sb[:, j * D + o2 * P: j * D + o2 * P + P], tp[:])

                nc.sync.dma_start(
                    t_sout[soff * P: soff * P + npad, :].rearrange(
                        "(j p) d -> p j d", p=P), e_sb)

                cc_in = dramp.tile([npad, D], F32, tag="cc", name="cc")
                nc.sync.dma_start(
                    cc_in[:].rearrange("(j p) d -> p j d", p=P), e_sb)
                nc.gpsimd.collective_compute(
                    "AllGather", mybir.AluOpType.bypass,
                    replica_groups=groups,
                    ins=[cc_in[:]],
                    outs=[buffer[blk: blk + NCORES * npad, :]])

                goff += 2 * k
                soff += k
                blk += NCORES * npad
    nc.compile()
    return nc


# ---------------------------------------------------------------------------
# entry point
# ---------------------------------------------------------------------------

_CACHE = {}


def _get_program(key, *args):
    if key not in _CACHE:
        _CACHE[key] = _build_program(*args)
    return _CACHE[key]


def kernel(root_node_embeddings, enc_W1, enc_b1, enc_W2, enc_b2,
           trunk_node_inputs_indices, trunk_node_types):
    from concourse import bass_utils

    root = np.ascontiguousarray(np.asarray(root_node_embeddings), dtype=np.float32)
    W1 = np.asarray(enc_W1, dtype=np.float32)
    W2 = np.asarray(enc_W2, dtype=np.float32)
    b1 = np.asarray(enc_b1, dtype=np.float32)
    b2 = np.asarray(enc_b2, dtype=np.float32)
    idx = np.asarray(trunk_node_inputs_indices)
    types = np.asarray(trunk_node_types)
    if types.ndim > 1:
        types = types[:, 0]
    types = types.astype(np.int64)
    idx64 = idx.astype(np.int64)
    n = idx64.shape[0]
    out_rows = R + n

    plan = _plan(idx64, types)
    level_ks = plan["level_ks"]
    key = (tuple(level_ks), plan["slots"], out_rows)
    nc = _get_program(key, level_ks, plan["slots"], plan["K"], out_rows)

    in_maps = []
    for c in range(NCORES):
        t = c // 2
        in_maps.append({
            "roots": root,
            "w1": np.ascontiguousarray(
                W1[t].reshape(4, P, 4, P).transpose(1, 0, 2, 3).reshape(P, 2048)),
            "w2": np.ascontiguousarray(
                W2[t].reshape(4, P, 2, P).transpose(1, 0, 2, 3).reshape(P, 1024)),
            "b1": np.ascontiguousarray(b1[t].reshape(4, P).T),
            "b2": np.ascontiguousarray(b2[t].reshape(2, P).T),
            "gidx": np.ascontiguousarray(plan["gidx"][c]),
        })

    res = bass_utils.run_bass_kernel_spmd(
        nc, in_maps, core_ids=list(range(NCORES)),
        trace=bool(int(os.environ.get("DAG_KERNEL_TRACE", "0"))))
    if res.exec_time_ns is not None:
        kernel.last_exec_time_ns = res.exec_time_ns

    out = np.zeros((out_rows, D), np.float32)
    out[:R] = root
    for c in range(NCORES):
        dst = plan["dst_rows"][c]
        if len(dst):
            out[dst] = res.results[c]["shard_out"][plan["src_rows"][c]]
    return out


kernel.last_exec_time_ns = None


# revision 7
# speedup vs baseline: 1.3191x; 1.3191x over previous
"""Trainium2 Bass kernel for the DagnabbitAutoEncoder DAG scan.

Strategy: the scan over 65536 trunk nodes is level-scheduled. Node depth =
1 + max(parent depths); the random DAG has only ~28 levels, and all nodes of
one level are independent.  Work is sharded across the 8 NeuronCores by node
TYPE (type t -> cores 2t, 2t+1), so each core's per-level shard uses a single
MLP weight set and its matmuls get a long free dimension.  The whole
datapath runs in fp16 (10-bit mantissa, same precision class as TF32;
fp32 PSUM accumulation), which keeps the PE at 1 cycle/row with fast weight
loads and halves all DMA / collective traffic.

Per level, per core:
  indirect-DMA gathers of the 2 parent embeddings for each shard node (from
  a fp16 DRAM embeddings buffer held in level-permuted order), PE-transpose
  to feature-major, 2-layer MLP (fp16 matmuls, exact-erf Gelu on ACT),
  PE-transpose back to node-major, then (a) plain DMA of the shard block
  into this core's contiguous shard_out output and (b) AllGather of the
  level block into every core's DRAM buffer for the next level.

Small trailing levels are processed entirely on core 0 ("solo" levels, all
four type weight sets resident), replacing the AllGather with a local
buffer write; the host takes those rows from core 0's output.

The host merges the 8 partial outputs (each core owns its shard rows).
"""

import math
import os

import numpy as np

R = 256
D = 256
NCORES = 8
P = 128
PSUM_N = 512
SOLO_MAX = 400  # max level size for solo (single-core) suffix levels


# ---------------------------------------------------------------------------
# host-side preprocessing
# ---------------------------------------------------------------------------

def _compute_levels(idx):
    n = idx.shape[0]
    depth = np.zeros(R + n, np.int32)
    ia = idx[:, 0]
    ib = idx[:, 1]
    d = depth
    for i in range(n):
        da = d[ia[i]]
        db = d[ib[i]]
        d[R + i] = (da if da > db else db) + 1
    return depth[R:]


def _plan(idx, types):
    n = idx.shape[0]
    lv = _compute_levels(idx)
    L = int(lv.max()) if n else 0
    order = np.argsort(lv, kind="stable")
    lv_sorted = lv[order]

    level_nodes = []
    lo = 0
    for l in range(1, L + 1):
        hi = lo + np.searchsorted(lv_sorted[lo:], l + 1)
        level_nodes.append(order[lo:hi])
        lo = hi

    # solo suffix: trailing levels small enough for one core
    solo_from = L
    while solo_from > 0 and len(level_nodes[solo_from - 1]) <= SOLO_MAX:
        solo_from -= 1

    pos = np.zeros(R + n, np.int64)
    pos[:R] = np.arange(R)
    slots = R
    specs = []   # per level: dict(k=..., mode=..., ranges=[(wblock,c0,c1)])
    shards = []  # per level: list of per-core node-id arrays
    for l in range(L):
        nodes = level_nodes[l]
        if l < solo_from:
            per_core = []
            for t in range(4):
                nt = nodes[types[nodes] == t]
                per_core.append(nt[0::2])
                per_core.append(nt[1::2])
            n_pad = max(len(s) for s in per_core)
            k = max(1, math.ceil(n_pad / P))
            pad = P * k
            for c in range(NCORES):
                s = per_core[c]
                pos[R + s] = slots + c * pad + np.arange(len(s))
            specs.append({"k": k, "mode": "split",
                          "ranges": [(4, 0, P * k)]})
            shards.append(per_core)
            slots += NCORES * pad
        else:
            # solo: all nodes on core 0, grouped by type
            groups = [nodes[types[nodes] == t] for t in range(4)]
            snodes = np.concatenate(groups)
            m = len(snodes)
            k = max(1, math.ceil(m / P))
            pos[R + snodes] = slots + np.arange(m)
            ranges = []
            c0 = 0
            for t in range(4):
                c1 = c0 + len(groups[t])
                if c1 > c0:
                    ranges.append((t, c0, c1))
                c0 = c1
            # all cores compute solo levels identically (replicated): local
            # buffer writes are then identical on every replica, which is
            # required both by the sim's Shared-tensor aliasing and to keep
            # every core's buffer complete without an AllGather.
            specs.append({"k": k, "mode": "solo", "ranges": ranges})
            shards.append([snodes] * NCORES)
            slots += P * k

    K = sum(s["k"] for s in specs)
    gidx = np.zeros((NCORES, P, 2 * K), np.int32)
    src_rows = [[] for _ in range(NCORES)]
    dst_rows = [[] for _ in range(NCORES)]
    goff = 0
    soff = 0
    for l in range(L):
        k = specs[l]["k"]
        solo = specs[l]["mode"] == "solo"
        for c in range(NCORES):
            s = shards[l][c]
            m = len(s)
            if m:
                sl = np.arange(m)
                pp = sl % P
                jj = sl // P
                gidx[c, pp, goff + 2 * jj] = pos[idx[s, 0]]
                gidx[c, pp, goff + 2 * jj + 1] = pos[idx[s, 1]]
                if not solo or c == 0:
                    src_rows[c].append(soff * P + sl)
                    dst_rows[c].append(R + s)
        goff += 2 * k
        soff += k
    src_rows = [np.concatenate(o) if o else np.zeros(0, np.int64) for o in src_rows]
    dst_rows = [np.concatenate(o) if o else np.zeros(0, np.int64) for o in dst_rows]
    return {
        "specs": specs,
        "slots": slots,
        "K": K,
        "gidx": gidx,
        "src_rows": src_rows,
        "dst_rows": dst_rows,
    }


# ---------------------------------------------------------------------------
# Bass program
# ---------------------------------------------------------------------------

def _build_program(specs, slots, K):
    import concourse.bass as bass
    import concourse.tile as tile
    from concourse import bacc, mybir
    from concourse.masks import make_identity

    F16 = mybir.dt.float16
    F32 = mybir.dt.float32
    I32 = mybir.dt.int32
    AF = mybir.ActivationFunctionType

    nc = bacc.Bacc("TRN2", target_bir_lowering=False, debug=False,
                   num_devices=NCORES)
    # weight block b in 0..3 = type b; block 4 = this core's own type (dup)
    t_root = nc.dram_tensor("roots", [R, D], F16, kind="ExternalInput")
    t_w1 = nc.dram_tensor("w1", [P, 5 * 2048], F16, kind="ExternalInput")
    t_w2 = nc.dram_tensor("w2", [P, 5 * 1024], F16, kind="ExternalInput")
    t_b1 = nc.dram_tensor("b1", [P, 5 * 4], F32, kind="ExternalInput")
    t_b2 = nc.dram_tensor("b2", [P, 5 * 2], F32, kind="ExternalInput")
    t_gidx = nc.dram_tensor("gidx", [P, 2 * K], I32, kind="ExternalInput")
    t_sout = nc.dram_tensor("shard_out", [P * K, D], F16, kind="ExternalOutput")
    buffer = nc.dram_tensor("buffer", [slots, D], F16, kind="Internal",
                            addr_space="Shared")
    groups = [list(range(NCORES))]
    k_max = max(s["k"] for s in specs)

    with tile.TileContext(nc) as tc:
        with (
            tc.tile_pool(name="const", bufs=1) as constp,
            tc.tile_pool(name="sbuf", bufs=2) as sbufp,
            tc.tile_pool(name="psum", bufs=1, space="PSUM") as psump,
            tc.tile_pool(name="dram", bufs=2, space="DRAM") as dramp,
        ):
            ident = constp.tile([P, P], F16)
            make_identity(nc, ident[:])
            w1_sb = constp.tile([P, 5 * 2048], F16)
            nc.sync.dma_start(w1_sb[:], t_w1[:])
            w2_sb = constp.tile([P, 5 * 1024], F16)
            nc.sync.dma_start(w2_sb[:], t_w2[:])
            b1_sb = constp.tile([P, 5 * 4], F32)
            nc.sync.dma_start(b1_sb[:], t_b1[:])
            b2_sb = constp.tile([P, 5 * 2], F32)
            nc.sync.dma_start(b2_sb[:], t_b2[:])
            gidx_sb = constp.tile([P, 2 * K], I32)
            nc.sync.dma_start(gidx_sb[:], t_gidx[:])

            # roots -> buffer[0:R]
            stg = sbufp.tile([P, (R // P) * D], F16, tag="stg")
            nc.sync.dma_start(
                stg[:], t_root[:].rearrange("(j p) d -> p j d", p=P))
            nc.sync.dma_start(
                buffer[0:R, :].rearrange("(j p) d -> p j d", p=P), stg[:])

            goff = 0
            soff = 0
            blk = R
            for l, spec in enumerate(specs):
                k = spec["k"]
                npad = P * k
                gx = sbufp.tile([P, 2 * k_max * D], F16, tag="gx",
                                name="gx")[:, : 2 * k * D]
                for col in range(2 * k):
                    nc.gpsimd.indirect_dma_start(
                        out=gx[:, col * D:(col + 1) * D], out_offset=None,
                        in_=buffer[:],
                        in_offset=bass.IndirectOffsetOnAxis(
                            ap=gidx_sb[:, goff + col: goff + col + 1], axis=0))

                xt = [sbufp.tile([P, P * k_max], F16, tag=f"xt{ic}",
                                 name=f"xt{ic}")[:, : npad] for ic in range(4)]
                for j in range(k):
                    for ic in range(4):
                        tp = psump.tile([P, P], F16, tag="tpose", bufs=2,
                                        name="tp")
                        nc.tensor.transpose(
                            tp[:],
                            gx[:, 2 * j * D + ic * P: 2 * j * D + (ic + 1) * P],
                            ident[:])
                        nc.vector.tensor_copy(xt[ic][:, j * P:(j + 1) * P],
                                              tp[:])

                h_sb = [sbufp.tile([P, P * k_max], F16, tag=f"h{oc}",
                                   name=f"h{oc}")[:, : npad] for oc in range(4)]
                et_sb = [sbufp.tile([P, P * k_max], F16, tag=f"et{o2}",
                                    name=f"et{o2}")[:, : npad] for o2 in range(2)]
                for g in range(math.ceil(npad / PSUM_N)):
                    g0 = g * PSUM_N
                    g1 = min(g0 + PSUM_N, npad)
                    for wb, r0, r1 in spec["ranges"]:
                        c0 = max(g0, r0)
                        c1 = min(g1, r1)
                        if c1 <= c0:
                            continue
                        ng = c1 - c0
                        cols = slice(c0, c1)
                        for oc in range(4):
                            hp = psump.tile([P, PSUM_N], F32,
                                            tag=f"hp{oc % 2}", name="hp")[:, :ng]
                            for ic in range(4):
                                w = w1_sb[:, wb * 2048 + ic * 512 + oc * P:
                                          wb * 2048 + ic * 512 + (oc + 1) * P]
                                nc.tensor.matmul(
                                    hp, lhsT=w, rhs=xt[ic][:, cols],
                                    start=(ic == 0), stop=(ic == 3))
                            nc.scalar.activation(
                                h_sb[oc][:, cols], hp, AF.Gelu,
                                bias=b1_sb[:, wb * 4 + oc: wb * 4 + oc + 1])
                        for o2 in range(2):
                            ep = psump.tile([P, PSUM_N], F32,
                                            tag=f"ep{o2}", name="ep")[:, :ng]
                            for ic in range(4):
                                w = w2_sb[:, wb * 1024 + ic * 256 + o2 * P:
                                          wb * 1024 + ic * 256 + (o2 + 1) * P]
                                nc.tensor.matmul(
                                    ep, lhsT=w, rhs=h_sb[ic][:, cols],
                                    start=(ic == 0), stop=(ic == 3))
                            nc.vector.tensor_add(
                                et_sb[o2][:, cols], ep,
                                b2_sb[:, wb * 2 + o2: wb * 2 + o2 + 1]
                                .to_broadcast([P, ng]))

                e_sb = sbufp.tile([P, k_max * D], F16, tag="e",
                                  name="e")[:, : k * D]
                for j in range(k):
                    for o2 in range(2):
                        tp = psump.tile([P, P], F16, tag="tpose", bufs=2,
                                        name="tp")
                        nc.tensor.transpose(
                            tp[:], et_sb[o2][:, j * P:(j + 1) * P], ident[:])
                        nc.vector.tensor_copy(
                            e_sb[:, j * D + o2 * P: j * D + o2 * P + P], tp[:])

                if spec["mode"] == "split":
                    cc_in = dramp.tile([npad, D], F16, tag="cc", name="cc")
                    nc.sync.dma_start(
                        cc_in[:].rearrange("(j p) d -> p j d", p=P), e_sb)
                    nc.gpsimd.collective_compute(
                        "AllGather", mybir.AluOpType.bypass,
                        replica_groups=groups,
                        ins=[cc_in[:]],
                        outs=[buffer[blk: blk + NCORES * npad, :]])
                    blk += NCORES * npad
                else:
                    for j in range(k):
                        nc.sync.dma_start(
                            buffer[blk + j * P: blk + (j + 1) * P, :],
                            e_sb[:, j * D:(j + 1) * D])
                    blk += npad
                nc.sync.dma_start(
                    t_sout[soff * P: soff * P + npad, :].rearrange(
                        "(j p) d -> p j d", p=P), e_sb)

                goff += 2 * k
                soff += k
    nc.compile()
    return nc


# ---------------------------------------------------------------------------
# entry point
# ---------------------------------------------------------------------------

_CACHE = {}


def _get_program(key, *args):
    if key not in _CACHE:
        _CACHE[key] = _build_program(*args)
    return _CACHE[key]


def kernel(root_node_embeddings, enc_W1, enc_b1, enc_W2, enc_b2,
           trunk_node_inputs_indices, trunk_node_types):
    from concourse import bass_utils

    root = np.asarray(root_node_embeddings, dtype=np.float32)
    W1 = np.asarray(enc_W1, dtype=np.float32)
    W2 = np.asarray(enc_W2, dtype=np.float32)
    b1 = np.asarray(enc_b1, dtype=np.float32)
    b2 = np.asarray(enc_b2, dtype=np.float32)
    idx = np.asarray(trunk_node_inputs_indices)
    types = np.asarray(trunk_node_types)
    if types.ndim > 1:
        types = types[:, 0]
    types = types.astype(np.int64)
    idx64 = idx.astype(np.int64)
    n = idx64.shape[0]

    plan = _plan(idx64, types)
    specs = plan["specs"]
    key = (tuple((s["k"], s["mode"], tuple(s["ranges"])) for s in specs),
           plan["slots"])
    nc = _get_program(key, specs, plan["slots"], plan["K"])

    def wtab(W, t):
        # [128, 5*width] fp16, blocks = [type0..type3, own-type]
        blocks = [W[b].reshape(4, P, -1, P).transpose(1, 0, 2, 3).reshape(P, -1)
                  for b in range(4)]
        blocks.append(blocks[t])
        return np.ascontiguousarray(np.concatenate(blocks, 1), dtype=np.float16)

    def btab(b, t, c):
        blocks = [b[bb].reshape(c, P).T for bb in range(4)]
        blocks.append(blocks[t])
        return np.ascontiguousarray(np.concatenate(blocks, 1), dtype=np.float32)

    in_maps = []
    for c in range(NCORES):
        t = c // 2
        in_maps.append({
            "roots": np.ascontiguousarray(root, dtype=np.float16),
            "w1": wtab(W1, t),
            "w2": wtab(W2, t),
            "b1": btab(b1, t, 4),
            "b2": btab(b2, t, 2),
            "gidx": np.ascontiguousarray(plan["gidx"][c]),
        })

    res = bass_utils.run_bass_kernel_spmd(
        nc, in_maps, core_ids=list(range(NCORES)),
        trace=bool(int(os.environ.get("DAG_KERNEL_TRACE", "0"))))
    if res.exec_time_ns is not None:
        kernel.last_exec_time_ns = res.exec_time_ns

    out = np.zeros((R + n, D), np.float32)
    out[:R] = root
    for c in range(NCORES):
        dst = plan["dst_rows"][c]
        if len(dst):
            out[dst] = res.results[c]["shard_out"][plan["src_rows"][c]].astype(
                np.float32)
    return out


kernel.last_exec_time_ns = None


# revision 8
# speedup vs baseline: 1.3403x; 1.0160x over previous
"""Trainium2 Bass kernel for the DagnabbitAutoEncoder DAG scan.

Strategy: the scan over 65536 trunk nodes is level-scheduled. Node depth =
1 + max(parent depths); the random DAG has only ~28 levels, and all nodes of
one level are independent.  Work is sharded across the 8 NeuronCores by node
TYPE (type t -> cores 2t, 2t+1), so each core's per-level shard uses a single
MLP weight set and its matmuls get a long free dimension.  The whole
datapath runs in fp16 (10-bit mantissa, same precision class as TF32;
fp32 PSUM accumulation), which keeps the PE at 1 cycle/row with fast weight
loads and halves all DMA / collective traffic.

Per level, per core:
  indirect-DMA gathers of the 2 parent embeddings for each shard node (from
  a fp16 DRAM embeddings buffer held in level-permuted order), PE-transpose
  to feature-major, 2-layer MLP (fp16 matmuls, exact-erf Gelu on ACT),
  PE-transpose back to node-major, then (a) plain DMA of the shard block
  into this core's contiguous shard_out output and (b) AllGather of the
  level block into every core's DRAM buffer for the next level.

Small trailing levels are processed entirely on core 0 ("solo" levels, all
four type weight sets resident), replacing the AllGather with a local
buffer write; the host takes those rows from core 0's output.

The host merges the 8 partial outputs (each core owns its shard rows).
"""

import math
import os

import numpy as np

R = 256
D = 256
NCORES = 8
P = 128
PSUM_N = 512
SOLO_MAX = 0    # solo levels disabled (per-type ranges bloat PE instr count)


# ---------------------------------------------------------------------------
# host-side preprocessing
# ---------------------------------------------------------------------------

def _compute_levels(idx):
    n = idx.shape[0]
    depth = np.zeros(R + n, np.int32)
    ia = idx[:, 0]
    ib = idx[:, 1]
    d = depth
    for i in range(n):
        da = d[ia[i]]
        db = d[ib[i]]
        d[R + i] = (da if da > db else db) + 1
    return depth[R:]


def _plan(idx, types):
    n = idx.shape[0]
    lv = _compute_levels(idx)
    L = int(lv.max()) if n else 0
    order = np.argsort(lv, kind="stable")
    lv_sorted = lv[order]

    level_nodes = []
    lo = 0
    for l in range(1, L + 1):
        hi = lo + np.searchsorted(lv_sorted[lo:], l + 1)
        level_nodes.append(order[lo:hi])
        lo = hi

    # solo suffix: trailing levels small enough for one core
    solo_from = L
    while solo_from > 0 and len(level_nodes[solo_from - 1]) <= SOLO_MAX:
        solo_from -= 1

    pos = np.zeros(R + n, np.int64)
    pos[:R] = np.arange(R)
    # level (1-based) of each buffer row; roots = 0
    rowlv = np.zeros(R + n, np.int64)
    rowlv[R:] = lv
    slots = R
    blk_starts = []  # permuted-buffer start row of each level's block
    specs = []   # per level: dict(k=..., mode=..., ranges=[(wblock,c0,c1)])
    shards = []  # per level: list of per-core node-id arrays
    for l in range(L):
        nodes = level_nodes[l]
        blk_starts.append(slots)
        if l < solo_from:
            per_core = []
            for t in range(4):
                nt = nodes[types[nodes] == t]
                per_core.append(nt[0::2])
                per_core.append(nt[1::2])
            for c in range(NCORES):
                s = per_core[c]
                # segment order: [parent0-old | parent1-old(only) | both-new]
                p0new = rowlv[idx[s, 0]] == l  # node level is l+1 (1-based)
                p1new = rowlv[idx[s, 1]] == l
                seg = np.where(~p0new, 0, np.where(~p1new, 1, 2))
                per_core[c] = s[np.argsort(seg, kind="stable")]
            n_pad = max(len(s) for s in per_core)
            k = max(1, math.ceil(n_pad / P))
            pad = P * k
            for c in range(NCORES):
                s = per_core[c]
                pos[R + s] = slots + c * pad + np.arange(len(s))
            specs.append({"k": k, "mode": "split",
                          "ranges": [(4, 0, P * k)]})
            shards.append(per_core)
            slots += NCORES * pad
        else:
            # solo: all nodes on core 0, grouped by type
            groups = [nodes[types[nodes] == t] for t in range(4)]
            snodes = np.concatenate(groups)
            m = len(snodes)
            k = max(1, math.ceil(m / P))
            pos[R + snodes] = slots + np.arange(m)
            ranges = []
            c0 = 0
            for t in range(4):
                c1 = c0 + len(groups[t])
                if c1 > c0:
                    ranges.append((t, c0, c1))
                c0 = c1
            # all cores compute solo levels identically (replicated): local
            # buffer writes are then identical on every replica, which is
            # required both by the sim's Shared-tensor aliasing and to keep
            # every core's buffer complete without an AllGather.
            specs.append({"k": k, "mode": "solo", "ranges": ranges})
            shards.append([snodes] * NCORES)
            slots += P * k

    K = sum(s["k"] for s in specs)
    gidx = np.zeros((NCORES, P, 2 * K), np.int32)
    src_rows = [[] for _ in range(NCORES)]
    dst_rows = [[] for _ in range(NCORES)]
    goff = 0
    soff = 0
    for l in range(L):
        k = specs[l]["k"]
        solo = specs[l]["mode"] == "solo"
        # a gather column is "early" (independent of the previous level's
        # AllGather) iff every index in it, on EVERY core, is < blk_starts[l]
        colmax = np.zeros(2 * k, np.int64)
        for c in range(NCORES):
            s = shards[l][c]
            m = len(s)
            if m:
                sl = np.arange(m)
                pp = sl % P
                jj = sl // P
                gidx[c, pp, goff + 2 * jj] = pos[idx[s, 0]]
                gidx[c, pp, goff + 2 * jj + 1] = pos[idx[s, 1]]
                np.maximum.at(colmax, 2 * jj, pos[idx[s, 0]])
                np.maximum.at(colmax, 2 * jj + 1, pos[idx[s, 1]])
                if not solo or c == 0:
                    src_rows[c].append(soff * P + sl)
                    dst_rows[c].append(R + s)
        prev_blk = blk_starts[l - 1] if l > 0 else R
        specs[l]["early"] = [bool(colmax[col] < prev_blk)
                             for col in range(2 * k)]
        specs[l]["early_bound"] = int(prev_blk)
        goff += 2 * k
        soff += k
    src_rows = [np.concatenate(o) if o else np.zeros(0, np.int64) for o in src_rows]
    dst_rows = [np.concatenate(o) if o else np.zeros(0, np.int64) for o in dst_rows]
    return {
        "specs": specs,
        "slots": slots,
        "K": K,
        "gidx": gidx,
        "src_rows": src_rows,
        "dst_rows": dst_rows,
    }


# ---------------------------------------------------------------------------
# Bass program
# ---------------------------------------------------------------------------

def _build_program(specs, slots, K):
    import concourse.bass as bass
    import concourse.tile as tile
    from concourse import bacc, mybir
    from concourse.masks import make_identity

    F16 = mybir.dt.float16
    F32 = mybir.dt.float32
    I32 = mybir.dt.int32
    AF = mybir.ActivationFunctionType

    nc = bacc.Bacc("TRN2", target_bir_lowering=False, debug=False,
                   num_devices=NCORES)
    # weight block b in 0..3 = type b; block 4 = this core's own type (dup)
    t_root = nc.dram_tensor("roots", [R, D], F16, kind="ExternalInput")
    t_w1 = nc.dram_tensor("w1", [P, 5 * 2048], F16, kind="ExternalInput")
    t_w2 = nc.dram_tensor("w2", [P, 5 * 1024], F16, kind="ExternalInput")
    t_b1 = nc.dram_tensor("b1", [P, 5 * 4], F32, kind="ExternalInput")
    t_b2 = nc.dram_tensor("b2", [P, 5 * 2], F32, kind="ExternalInput")
    t_gidx = nc.dram_tensor("gidx", [P, 2 * K], I32, kind="ExternalInput")
    t_sout = nc.dram_tensor("shard_out", [P * K, D], F16, kind="ExternalOutput")
    buffer = nc.dram_tensor("buffer", [slots, D], F16, kind="Internal",
                            addr_space="Shared")
    groups = [list(range(NCORES))]
    k_max = max(s["k"] for s in specs)

    with tile.TileContext(nc) as tc:
        with (
            tc.tile_pool(name="const", bufs=1) as constp,
            tc.tile_pool(name="sbuf", bufs=2) as sbufp,
            tc.tile_pool(name="psum", bufs=1, space="PSUM") as psump,
            tc.tile_pool(name="dram", bufs=2, space="DRAM") as dramp,
        ):
            ident = constp.tile([P, P], F16)
            make_identity(nc, ident[:])
            w1_sb = constp.tile([P, 5 * 2048], F16)
            nc.sync.dma_start(w1_sb[:], t_w1[:])
            w2_sb = constp.tile([P, 5 * 1024], F16)
            nc.sync.dma_start(w2_sb[:], t_w2[:])
            b1_sb = constp.tile([P, 5 * 4], F32)
            nc.sync.dma_start(b1_sb[:], t_b1[:])
            b2_sb = constp.tile([P, 5 * 2], F32)
            nc.sync.dma_start(b2_sb[:], t_b2[:])
            gidx_sb = constp.tile([P, 2 * K], I32)
            nc.sync.dma_start(gidx_sb[:], t_gidx[:])

            # roots -> buffer[0:R]
            stg = sbufp.tile([P, (R // P) * D], F16, tag="stg")
            nc.sync.dma_start(
                stg[:], t_root[:].rearrange("(j p) d -> p j d", p=P))
            nc.sync.dma_start(
                buffer[0:R, :].rearrange("(j p) d -> p j d", p=P), stg[:])

            goff = 0
            soff = 0
            blk = R
            for l, spec in enumerate(specs):
                k = spec["k"]
                npad = P * k
                gx = sbufp.tile([P, 2 * k_max * D], F16, tag="gx",
                                name="gx")[:, : 2 * k * D]
                early = spec["early"]
                bound = spec["early_bound"]
                cols_order = ([c_ for c_ in range(2 * k) if early[c_]] +
                              [c_ for c_ in range(2 * k) if not early[c_]])
                for col in cols_order:
                    src = buffer[0:bound, :] if early[col] else buffer[:]
                    nc.gpsimd.indirect_dma_start(
                        out=gx[:, col * D:(col + 1) * D], out_offset=None,
                        in_=src,
                        in_offset=bass.IndirectOffsetOnAxis(
                            ap=gidx_sb[:, goff + col: goff + col + 1], axis=0))

                xt = [sbufp.tile([P, P * k_max], F16, tag=f"xt{ic}",
                                 name=f"xt{ic}")[:, : npad] for ic in range(4)]
                tjobs = [(j, ic) for j in range(k) for ic in range(4)]
                tjobs.sort(key=lambda ji: not early[2 * ji[0] + ji[1] // 2])
                for j, ic in tjobs:
                    tp = psump.tile([P, P], F16, tag="tpose", bufs=2,
                                    name="tp")
                    nc.tensor.transpose(
                        tp[:],
                        gx[:, 2 * j * D + ic * P: 2 * j * D + (ic + 1) * P],
                        ident[:])
                    nc.vector.tensor_copy(xt[ic][:, j * P:(j + 1) * P],
                                          tp[:])

                h_sb = [sbufp.tile([P, P * k_max], F16, tag=f"h{oc}",
                                   name=f"h{oc}")[:, : npad] for oc in range(4)]
                et_sb = [sbufp.tile([P, P * k_max], F16, tag=f"et{o2}",
                                    name=f"et{o2}")[:, : npad] for o2 in range(2)]
                for g in range(math.ceil(npad / PSUM_N)):
                    g0 = g * PSUM_N
                    g1 = min(g0 + PSUM_N, npad)
                    for wb, r0, r1 in spec["ranges"]:
                        c0 = max(g0, r0)
                        c1 = min(g1, r1)
                        if c1 <= c0:
                            continue
                        ng = c1 - c0
                        cols = slice(c0, c1)
                        for oc in range(4):
                            hp = psump.tile([P, PSUM_N], F32,
                                            tag=f"hp{oc % 2}", name="hp")[:, :ng]
                            for ic in range(4):
                                w = w1_sb[:, wb * 2048 + ic * 512 + oc * P:
                                          wb * 2048 + ic * 512 + (oc + 1) * P]
                                nc.tensor.matmul(
                                    hp, lhsT=w, rhs=xt[ic][:, cols],
                                    start=(ic == 0), stop=(ic == 3))
                            nc.scalar.activation(
                                h_sb[oc][:, cols], hp, AF.Gelu,
                                bias=b1_sb[:, wb * 4 + oc: wb * 4 + oc + 1])
                        for o2 in range(2):
                            ep = psump.tile([P, PSUM_N], F32,
                                            tag=f"ep{o2}", name="ep")[:, :ng]
                            for ic in range(4):
                                w = w2_sb[:, wb * 1024 + ic * 256 + o2 * P:
                                          wb * 1024 + ic * 256 + (o2 + 1) * P]
                                nc.tensor.matmul(
                                    ep, lhsT=w, rhs=h_sb[ic][:, cols],
                                    start=(ic == 0), stop=(ic == 3))
                            nc.vector.tensor_add(
                                et_sb[o2][:, cols], ep,
                                b2_sb[:, wb * 2 + o2: wb * 2 + o2 + 1]
                                .to_broadcast([P, ng]))

                e_sb = sbufp.tile([P, k_max * D], F16, tag="e",
                                  name="e")[:, : k * D]
                for j in range(k):
                    for o2 in range(2):
                        tp = psump.tile([P, P], F16, tag="tpose", bufs=2,
                                        name="tp")
                        nc.tensor.transpose(
                            tp[:], et_sb[o2][:, j * P:(j + 1) * P], ident[:])
                        nc.vector.tensor_copy(
                            e_sb[:, j * D + o2 * P: j * D + o2 * P + P], tp[:])

                if spec["mode"] == "split":
                    cc_in = dramp.tile([npad, D], F16, tag="cc", name="cc")
                    nc.sync.dma_start(
                        cc_in[:].rearrange("(j p) d -> p j d", p=P), e_sb)
                    nc.gpsimd.collective_compute(
                        "AllGather", mybir.AluOpType.bypass,
                        replica_groups=groups,
                        ins=[cc_in[:]],
                        outs=[buffer[blk: blk + NCORES * npad, :]])
                    blk += NCORES * npad
                else:
                    for j in range(k):
                        nc.sync.dma_start(
                            buffer[blk + j * P: blk + (j + 1) * P, :],
                            e_sb[:, j * D:(j + 1) * D])
                    blk += npad
                nc.sync.dma_start(
                    t_sout[soff * P: soff * P + npad, :].rearrange(
                        "(j p) d -> p j d", p=P), e_sb)

                goff += 2 * k
                soff += k
    nc.compile()
    return nc


# ---------------------------------------------------------------------------
# entry point
# ---------------------------------------------------------------------------

_CACHE = {}


def _get_program(key, *args):
    if key not in _CACHE:
        _CACHE[key] = _build_program(*args)
    return _CACHE[key]


def kernel(root_node_embeddings, enc_W1, enc_b1, enc_W2, enc_b2,
           trunk_node_inputs_indices, trunk_node_types):
    from concourse import bass_utils

    root = np.asarray(root_node_embeddings, dtype=np.float32)
    W1 = np.asarray(enc_W1, dtype=np.float32)
    W2 = np.asarray(enc_W2, dtype=np.float32)
    b1 = np.asarray(enc_b1, dtype=np.float32)
    b2 = np.asarray(enc_b2, dtype=np.float32)
    idx = np.asarray(trunk_node_inputs_indices)
    types = np.asarray(trunk_node_types)
    if types.ndim > 1:
        types = types[:, 0]
    types = types.astype(np.int64)
    idx64 = idx.astype(np.int64)
    n = idx64.shape[0]

    plan = _plan(idx64, types)
    specs = plan["specs"]
    key = (tuple((s["k"], s["mode"], tuple(s["ranges"])) for s in specs),
           plan["slots"])
    nc = _get_program(key, specs, plan["slots"], plan["K"])

    def wtab(W, t):
        # [128, 5*width] fp16, blocks = [type0..type3, own-type]
        blocks = [W[b].reshape(4, P, -1, P).transpose(1, 0, 2, 3).reshape(P, -1)
                  for b in range(4)]
        blocks.append(blocks[t])
        return np.ascontiguousarray(np.concatenate(blocks, 1), dtype=np.float16)

    def btab(b, t, c):
        blocks = [b[bb].reshape(c, P).T for bb in range(4)]
        blocks.append(blocks[t])
        return np.ascontiguousarray(np.concatenate(blocks, 1), dtype=np.float32)

    in_maps = []
    for c in range(NCORES):
        t = c // 2
        in_maps.append({
            "roots": np.ascontiguousarray(root, dtype=np.float16),
            "w1": wtab(W1, t),
            "w2": wtab(W2, t),
            "b1": btab(b1, t, 4),
            "b2": btab(b2, t, 2),
            "gidx": np.ascontiguousarray(plan["gidx"][c]),
        })

    res = bass_utils.run_bass_kernel_spmd(
        nc, in_maps, core_ids=list(range(NCORES)),
        trace=bool(int(os.environ.get("DAG_KERNEL_TRACE", "0"))))
    if res.exec_time_ns is not None:
        kernel.last_exec_time_ns = res.exec_time_ns

    out = np.zeros((R + n, D), np.float32)
    out[:R] = root
    for c in range(NCORES):
        dst = plan["dst_rows"][c]
        if len(dst):
            out[dst] = res.results[c]["shard_out"][plan["src_rows"][c]].astype(
                np.float32)
    return out


kernel.last_exec_time_ns = None
